# revision 25
# baseline (speedup 1.0000x reference)
"""Causal self-attention (B=2, T=4096, C=768, H=12, D=64) on 8 trn2 cores.

Sharding: batch*heads across cores. Core c handles batch c//4 and heads
3*(c%4) .. 3*(c%4)+2. Each core computes the QKV projection for its head
slice, full causal attention for those heads, and a partial output
projection (its heads' rows of w_out). The host sums the 4 partials per
batch and adds b_out.

On-core layouts (q/k matmul operands float32r - fp32 consumed at full PE
rate with ~1e-4 rounding; v / softmax weights / out-proj in bf16):
  xT      [C, T]    input, pre-transposed on host
  tA      [128, T]  = [qT_h0 | qT_h1]   (rows 0:64 | 64:128)
  tB      [128, T]  = [kT_h0 | kT_h1]
  tQ2/tK2 [128, T]  = [qT_h2 | qT_h2] / [kT_h2 | kT_h2]  (duplicated
                      halves so h2's scores can use either partition
                      base without a cross-partition copy)
  vaug    [128, NKT, 256] bf16, v with a ones column per head at
                      col h*65+64 (so P@V also yields softmax denoms)
  scores  [128, 1024] psum, TWO k-tiles per bank-pair so one ACT exp
                      instruction covers 1024 columns (amortizes the
                      ~350-cycle ACT instruction overhead)
  attnO   [64, 3, T] bf16 normalized attention outputs per head

h0 lives at partition base 0 and h1 at base 64, and their score matmuls
are emitted adjacently, so the PE can run them concurrently in disjoint
row-groups (K=64 each).

Softmax denominators: PV psum row 64 = sum(exp) -> DVE reciprocal ->
gpsimd partition_broadcast -> DVE multiply. No DRAM round trips.

The causal mask is applied to the exp output of the 4 diagonal k-tiles
per q-block with gpsimd affine_select (fills 0 where q < k, including
the never-computed stale region of the staircase).
"""

import numpy as np
from ml_dtypes import bfloat16

import concourse.bass as bass
import concourse.mybir as mybir
import concourse.tile as tile
from concourse import bacc
from concourse.bass_utils import run_bass_kernel_spmd

B, T, C = 2, 4096, 768
NH, D = 12, 64
HPC = 3  # heads per core
NCORES = 8
P = 128
QB = 512           # q block == projection chunk
NQB = T // QB      # 8
NKT = T // P       # 32 k tiles
GK = 2             # k-tiles per exp group (1 or 2)
F32 = mybir.dt.float32
F32R = mybir.dt.float32r
BF16 = mybir.dt.bfloat16

_CACHE = {}


def _declare(nc):
    xT = nc.dram_tensor("xT", [C, T], BF16, kind="ExternalInput")
    wqk = nc.dram_tensor("wqk", [C, 3 * P], BF16, kind="ExternalInput")
    wv = nc.dram_tensor("wv", [C, 256], BF16, kind="ExternalInput")
    wo = nc.dram_tensor("wo", [HPC * D, C], BF16, kind="ExternalInput")
    out = nc.dram_tensor("out", [T, C], BF16, kind="ExternalOutput")
    return dict(xT=xT.ap(), wqk=wqk.ap(), wv=wv.ap(), wo=wo.ap(), out=out.ap())


def _build_nc():
    nc = bacc.Bacc(
        "TRN2",
        target_bir_lowering=False,
        debug=False,
        enable_asserts=False,
        num_devices=NCORES,
    )
    aps = _declare(nc)
    with tile.TileContext(nc) as tc:
        _emit(tc, nc, **aps)
    nc.compile()
    return nc


def _emit(tc, nc, xT, wqk, wv, wo, out):
    import contextlib

    ctx = contextlib.ExitStack()
    with ctx:
        # ---- persistent sbuf ----
        persist = ctx.enter_context(tc.tile_pool(name="persist", bufs=1))
        tA = persist.tile([P, T], BF16, tag="tA")
        tB = persist.tile([P, T], BF16, tag="tB")
        tD = persist.tile([P, T], BF16, tag="tD")   # rows 64:128 = qT_h2
        tE = persist.tile([P, T], BF16, tag="tE")   # rows 64:128 = kT_h2
        vaug = persist.tile([P, NKT, 256], BF16, tag="vaug")
        attnOAB = persist.tile([P, T], BF16, tag="attnOAB")
        attnO2 = persist.tile([D, T], BF16, tag="attnO2")
        wqk_sb = persist.tile([P, 6, 3 * P], BF16, tag="wqk")
        wv_sb = persist.tile([P, 6, 256], BF16, tag="wv")
        woAB_sb = persist.tile([P, C], BF16, tag="woAB")
        wo2_sb = persist.tile([D, C], BF16, tag="wo2")
        ones_bf = persist.tile([P, 8], BF16, tag="ones")

        nc.sync.dma_start(out=wqk_sb[:], in_=wqk.rearrange("(co p) n -> p co n", p=P))
        nc.sync.dma_start(out=wv_sb[:], in_=wv.rearrange("(co p) n -> p co n", p=P))
        nc.sync.dma_start(out=woAB_sb[:], in_=wo[0:P, :])
        nc.sync.dma_start(out=wo2_sb[:], in_=wo[P : P + D, :])
        nc.gpsimd.memset(ones_bf[:], 1.0)

        def qT(h, base=0):
            # h0 rows 0:64 of tA, h1 rows 64:128 of tA, h2 rows 64:128 of tD
            return (tA[0:D], tA[D:P], tD[D:P])[h]

        def kT(h, base=0):
            return (tB[0:D], tB[D:P], tE[D:P])[h]

        # psum budget (8 banks): p1 2 + scores 2*2 + outT 2
        with (
            tc.tile_pool(name="xchunks", bufs=3) as xpool,
            tc.tile_pool(name="p1psum", bufs=2, space="PSUM") as p1psum,
            tc.tile_pool(name="spsum", bufs=4 // GK, space="PSUM") as spool,
            tc.tile_pool(name="opsum", bufs=2, space="PSUM") as opool,
            tc.tile_pool(name="exps", bufs=4) as epool,
            tc.tile_pool(name="smalls", bufs=4) as rpool,
        ):
            from collections import deque

            def proj_work(qb):
                """Closures emitting projection chunk qb (tokens qb*512..)."""
                qsl = slice(qb * QB, (qb + 1) * QB)
                st = {}

                def dma():
                    xt = xpool.tile([P, 6, QB], BF16, tag="xt")
                    nc.sync.dma_start(
                        out=xt[:],
                        in_=xT[:, qsl].rearrange("(co p) t -> p co t", p=P),
                    )
                    st["xt"] = xt

                def chain(ci):
                    def f():
                        ps = p1psum.tile(
                            [P, QB], F32, tag="p1", name=f"p1_{qb}_{ci}"
                        )
                        for c6 in range(6):
                            nc.tensor.matmul(
                                ps[:],
                                wqk_sb[:, c6, ci * P : (ci + 1) * P],
                                st["xt"][:, c6, :],
                                start=(c6 == 0),
                                stop=(c6 == 5),
                            )
                        if ci < 2:
                            nc.vector.tensor_copy(
                                out=(tA, tB)[ci][:, qsl], in_=ps[:]
                            )
                        else:
                            # [q2|k2]: k2 (rows 64:) straight to tE; q2 must
                            # cross partitions - stage + DMA bounce
                            nc.vector.tensor_copy(
                                out=tE[D:P, qsl], in_=ps[D:P, :]
                            )
                            stg = rpool.tile([D, QB], BF16, tag="stg",
                                             bufs=2, name=f"stg_{qb}")
                            nc.vector.tensor_copy(out=stg[:], in_=ps[0:D, :])
                            nc.sync.dma_start(out=tD[D:P, qsl], in_=stg[:])
                    return f

                def vhalf(half):
                    def f():
                        ktv = qb * (QB // P) + half
                        ps2 = p1psum.tile(
                            [P, QB], F32, tag="p1", name=f"p1v_{qb}_{half}"
                        )
                        for c6 in range(6):
                            nc.tensor.matmul(
                                ps2[:, 0:256],
                                st["xt"][:, c6, half * P : (half + 1) * P],
                                wv_sb[:, c6, :],
                                start=(c6 == 0),
                                stop=(c6 == 5),
                            )
                        nc.vector.tensor_copy(
                            out=vaug[:, ktv, :], in_=ps2[:, 0:256]
                        )
                        if half == QB // P - 1:
                            # restore the ones columns the v copies overwrote
                            for h in range(HPC):
                                nc.vector.tensor_copy(
                                    out=vaug[:, qb * (QB // P) :
                                             (qb + 1) * (QB // P),
                                             h * (D + 1) + D],
                                    in_=ones_bf[:, 0 : QB // P],
                                )
                    return f

                return (
                    [dma]
                    + [chain(ci) for ci in range(3)]
                    + [vhalf(h) for h in range(QB // P)]
                )

            def outproj_work(qb):
                """Closures emitting the output projection of q block qb."""
                def tt_work(tt):
                    def f():
                        tsl = slice(tt * P, (tt + 1) * P)
                        so = rpool.tile([P, C], BF16, tag="p3out", bufs=2)
                        for noff, nsz in ((0, 512), (512, 256)):
                            po = p1psum.tile(
                                [P, QB], F32, tag="p1", name=f"po_{tt}_{noff}"
                            )
                            nc.tensor.matmul(
                                po[:, :nsz],
                                attnOAB[:, tsl],
                                woAB_sb[:, noff : noff + nsz],
                                start=True,
                                stop=False,
                            )
                            nc.tensor.matmul(
                                po[:, :nsz],
                                attnO2[:, tsl],
                                wo2_sb[:, noff : noff + nsz],
                                start=False,
                                stop=True,
                            )
                            nc.vector.tensor_copy(
                                out=so[:, noff : noff + nsz], in_=po[:, :nsz]
                            )
                        nc.sync.dma_start(out=out[tsl, :], in_=so[:])
                    return f

                return [
                    tt_work(tt)
                    for tt in range(qb * (QB // P), (qb + 1) * (QB // P))
                ]

            def attn_group(qb, g, hb, outps):
                """Two k-tiles (2g, 2g+1) for one or two heads: interleaved
                scores (consecutive matmuls always hit alternating PE
                row-groups, which run concurrently; same-row-group
                back-to-back K=64 matmuls serialize their weight loads and
                cost ~2.3x) -> one exp per head -> mask -> PV."""
                nkt = 4 * qb + 4
                hb = list(hb)
                sps, exs = [], []
                for h, _ in hb:
                    sps.append(spool.tile([P, GK * QB], F32, tag="sc",
                                          name=f"sp_{qb}_{h}_{g}"))
                for i in range(GK):
                    kt = GK * g + i
                    for (h, bases), sp in zip(hb, sps):
                        base = bases[i]
                        nc.tensor.matmul(
                            sp[:, i * QB : (i + 1) * QB],
                            kT(h, base)[:, kt * P : (kt + 1) * P],
                            qT(h, base)[:, qb * QB : (qb + 1) * QB],
                            start=True,
                            stop=True,
                        )
                for sp in sps:
                    ex = epool.tile([P, GK * QB], BF16, tag="ex")
                    nc.scalar.activation(
                        out=ex[:],
                        in_=sp[:],
                        func=mybir.ActivationFunctionType.Exp,
                        scale=float(D) ** -0.5,
                    )
                    exs.append(ex)
                for i in range(GK):
                    kt = GK * g + i
                    j = kt - 4 * qb
                    if j >= 0:  # diagonal tile: causal mask (fill 0 at q<k)
                        for ex in exs:
                            nc.gpsimd.affine_select(
                                out=ex[:, i * QB : (i + 1) * QB],
                                in_=ex[:, i * QB : (i + 1) * QB],
                                compare_op=mybir.AluOpType.is_ge,
                                fill=0.0,
                                base=-P * j,
                                pattern=[[1, QB]],
                                channel_multiplier=-1,
                            )
                for ((h, _), ex, outp) in zip(hb, exs, outps):
                    for i in range(GK):
                        kt = GK * g + i
                        co = max(0, P * (kt - 4 * qb))
                        nc.tensor.matmul(
                            outp[:, co:],
                            vaug[:, kt, h * (D + 1) : (h + 1) * (D + 1)],
                            ex[:, i * QB + co : (i + 1) * QB],
                            start=(kt == 0),
                            stop=(kt == nkt - 1),
                        )

            def normalize(qb, hs, outps):
                """Softmax denominators for one or two heads at once: DVE
                reciprocal of psum row 64, one sbuf->sbuf DMA to move the
                rows to partition 0 (partition_broadcast's ucode reads
                physical partition 0 only), one gpsimd broadcast, then the
                normalizing multiplies."""
                qsl = slice(qb * QB, (qb + 1) * QB)
                nh = len(hs)
                # copy PV psum to sbuf promptly so the opsum bank frees for
                # the next head's accumulation; normalize runs off sbuf
                ot = rpool.tile([D + 1, nh, QB], F32, tag=f"ot{nh}", bufs=2)
                for i, outp in enumerate(outps):
                    nc.vector.tensor_copy(out=ot[:, i, :], in_=outp[:])
                rt = rpool.tile([D + 1, nh, QB], F32, tag=f"recip{nh}", bufs=1)
                nc.vector.reciprocal(
                    out=rt[D : D + 1, :, :], in_=ot[D : D + 1, :, :]
                )
                rb = rpool.tile([1, nh, QB], F32, tag=f"rb{nh}", bufs=1)
                nc.sync.dma_start(out=rb[:], in_=rt[D : D + 1, :, :])
                rbc = rpool.tile([D, nh, QB], F32, tag=f"rbc{nh}", bufs=1)
                nc.gpsimd.partition_broadcast(rbc[:], rb[:])
                for i, h in enumerate(hs):
                    if h == 0:
                        dst = attnOAB[0:D, qsl]
                    elif h == 2:
                        dst = attnO2[:, qsl]
                    else:
                        # h1 belongs at partitions 64:128 of attnOAB, which
                        # engines cannot reach from lanes 0:64 - stage and
                        # DMA-bounce (sbuf->sbuf)
                        sg = rpool.tile([D, QB], BF16, tag="sg", bufs=2,
                                        name=f"sg_{qb}")
                        dst = sg[:]
                    nc.vector.tensor_mul(
                        out=dst, in0=ot[0:D, i, :], in1=rbc[:, i, :]
                    )
                    if h == 1:
                        nc.sync.dma_start(out=attnOAB[D:P, qsl], in_=dst)

            # Software pipeline by emission order: attention(qb) interleaves
            # closures of outproj(qb-1) and proj(qb+1) between its groups so
            # the PE stream stays dense and no sequencer head-of-line blocks.
            for w in proj_work(0):
                w()
            pending = deque(proj_work(1) if NQB > 1 else [])
            for qb in range(NQB):
                nkt = 4 * qb + 4
                # h0 (base 0) and h1 (base 64) interleaved for PE row-group
                # concurrency; h2 follows solo with alternating base
                outp0 = opool.tile([D + 1, QB], F32, tag="outT", name=f"o0_{qb}")
                outp1 = opool.tile([D + 1, QB], F32, tag="outT", name=f"o1_{qb}")
                for g in range(nkt // GK):
                    attn_group(qb, g, [(0, (0, 0)), (1, (0, 0))],
                               [outp0, outp1])
                    if pending:
                        pending.popleft()()
                normalize(qb, (0, 1), (outp0, outp1))
                outp2 = opool.tile([D + 1, QB], F32, tag="outT", name=f"o2_{qb}")
                for g in range(nkt // GK):
                    attn_group(qb, g, [(2, (D, D))], [outp2])
                    if pending:
                        pending.popleft()()
                normalize(qb, (2,), (outp2,))
                while pending:
                    pending.popleft()()
                pending = deque(outproj_work(qb))
                if qb + 2 < NQB:
                    pending.extend(proj_work(qb + 2))
            while pending:
                pending.popleft()()


def _get_nc():
    if "nc" not in _CACHE:
        _CACHE["nc"] = _build_nc()
    return _CACHE["nc"]


def _shard_inputs(x, w_qkv, w_out):
    """Build per-core input maps."""
    x = np.asarray(x, dtype=np.float32)
    w_qkv = np.asarray(w_qkv, dtype=np.float32)
    w_out = np.asarray(w_out, dtype=np.float32)
    xTs = [np.ascontiguousarray(x[b].T) for b in range(B)]
    in_maps = []
    for c in range(NCORES):
        b = c // 4
        heads = [HPC * (c % 4) + i for i in range(HPC)]
        q = [w_qkv[:, h * D : (h + 1) * D] for h in heads]
        k = [w_qkv[:, C + h * D : C + (h + 1) * D] for h in heads]
        wqk = np.concatenate([q[0], q[1], k[0], k[1], q[2], k[2]], axis=1)
        wv = np.zeros((C, 256), dtype=np.float32)
        for i, h in enumerate(heads):
            wv[:, i * (D + 1) : i * (D + 1) + D] = w_qkv[
                :, 2 * C + h * D : 2 * C + (h + 1) * D
            ]
        wo = np.concatenate(
            [w_out[h * D : (h + 1) * D, :] for h in heads], axis=0
        )  # [HPC*D, C]
        in_maps.append(
            {
                "xT": xTs[b].astype(bfloat16),
                "wqk": np.ascontiguousarray(wqk).astype(bfloat16),
                "wv": wv.astype(bfloat16),
                "wo": np.ascontiguousarray(wo).astype(bfloat16),
            }
        )
    return in_maps


def kernel(x, w_qkv, w_out, b_out):
    nc = _get_nc()
    in_maps = _shard_inputs(x, w_qkv, w_out)
    res = run_bass_kernel_spmd(nc, in_maps, core_ids=list(range(NCORES)))
    b_out = np.asarray(b_out, dtype=np.float32)
    outs = []
    for b in range(B):
        acc = res.results[4 * b]["out"].astype(np.float32).copy()
        for c in range(4 * b + 1, 4 * b + 4):
            acc += res.results[c]["out"]
        outs.append(acc + b_out[None, :])
    return np.stack(outs, axis=0)


# revision 26
# speedup vs baseline: 1.1497x; 1.1497x over previous
"""Causal self-attention (B=2, T=4096, C=768, H=12, D=64) on 8 trn2 cores.

Sharding: batch*heads across cores. Core c handles batch c//4 and heads
3*(c%4) .. 3*(c%4)+2. Each core computes the QKV projection for its head
slice, full causal attention for those heads, and a partial output
projection (its heads' rows of w_out). The host sums the 4 partials per
batch and adds b_out.

On-core layouts (q/k matmul operands float32r - fp32 consumed at full PE
rate with ~1e-4 rounding; v / softmax weights / out-proj in bf16):
  xT      [C, T]    input, pre-transposed on host
  tA      [128, T]  = [qT_h0 | qT_h1]   (rows 0:64 | 64:128)
  tB      [128, T]  = [kT_h0 | kT_h1]
  tQ2/tK2 [128, T]  = [qT_h2 | qT_h2] / [kT_h2 | kT_h2]  (duplicated
                      halves so h2's scores can use either partition
                      base without a cross-partition copy)
  vaug    [128, NKT, 256] bf16, v with a ones column per head at
                      col h*65+64 (so P@V also yields softmax denoms)
  scores  [128, 1024] psum, TWO k-tiles per bank-pair so one ACT exp
                      instruction covers 1024 columns (amortizes the
                      ~350-cycle ACT instruction overhead)
  attnO   [64, 3, T] bf16 normalized attention outputs per head

h0 lives at partition base 0 and h1 at base 64, and their score matmuls
are emitted adjacently, so the PE can run them concurrently in disjoint
row-groups (K=64 each).

Softmax denominators: PV psum row 64 = sum(exp) -> DVE reciprocal ->
gpsimd partition_broadcast -> DVE multiply. No DRAM round trips.

The causal mask is applied to the exp output of the 4 diagonal k-tiles
per q-block with gpsimd affine_select (fills 0 where q < k, including
the never-computed stale region of the staircase).
"""

import numpy as np
from ml_dtypes import bfloat16

import concourse.bass as bass
import concourse.mybir as mybir
import concourse.tile as tile
from concourse import bacc
from concourse.bass_utils import run_bass_kernel_spmd

B, T, C = 2, 4096, 768
NH, D = 12, 64
HPC = 3  # heads per core
NCORES = 8
P = 128
QB = 512           # q block == projection chunk
NQB = T // QB      # 8
NKT = T // P       # 32 k tiles
GK = 2             # k-tiles per exp group (1 or 2)
F32 = mybir.dt.float32
F32R = mybir.dt.float32r
BF16 = mybir.dt.bfloat16

_CACHE = {}


def _declare(nc):
    xT = nc.dram_tensor("xT", [C, T], BF16, kind="ExternalInput")
    wqk = nc.dram_tensor("wqk", [C, 3 * P], BF16, kind="ExternalInput")
    wv = nc.dram_tensor("wv", [C, 256], BF16, kind="ExternalInput")
    wo = nc.dram_tensor("wo", [HPC * D, C], BF16, kind="ExternalInput")
    out = nc.dram_tensor("out", [T, C], BF16, kind="ExternalOutput")
    return dict(xT=xT.ap(), wqk=wqk.ap(), wv=wv.ap(), wo=wo.ap(), out=out.ap())


def _build_nc():
    nc = bacc.Bacc(
        "TRN2",
        target_bir_lowering=False,
        debug=False,
        enable_asserts=False,
        num_devices=NCORES,
    )
    aps = _declare(nc)
    with tile.TileContext(nc) as tc:
        _emit(tc, nc, **aps)
    nc.compile()
    return nc


def _emit(tc, nc, xT, wqk, wv, wo, out):
    import contextlib

    ctx = contextlib.ExitStack()
    with ctx:
        # ---- persistent sbuf ----
        persist = ctx.enter_context(tc.tile_pool(name="persist", bufs=1))
        tA = persist.tile([P, T], BF16, tag="tA")
        tB = persist.tile([P, T], BF16, tag="tB")
        tD = persist.tile([P, T], BF16, tag="tD")   # rows 64:128 = qT_h2
        tE = persist.tile([P, T], BF16, tag="tE")   # rows 64:128 = kT_h2
        vaug = persist.tile([P, NKT, 256], BF16, tag="vaug")
        attnOAB = persist.tile([P, T], BF16, tag="attnOAB")
        attnO2 = persist.tile([D, T], BF16, tag="attnO2")
        wqk_sb = persist.tile([P, 6, 3 * P], BF16, tag="wqk")
        wv_sb = persist.tile([P, 6, 256], BF16, tag="wv")
        woAB_sb = persist.tile([P, C], BF16, tag="woAB")
        wo2_sb = persist.tile([D, C], BF16, tag="wo2")
        ones_bf = persist.tile([P, 8], BF16, tag="ones")

        nc.sync.dma_start(out=wqk_sb[:], in_=wqk.rearrange("(co p) n -> p co n", p=P))
        nc.sync.dma_start(out=wv_sb[:], in_=wv.rearrange("(co p) n -> p co n", p=P))
        nc.sync.dma_start(out=woAB_sb[:], in_=wo[0:P, :])
        nc.sync.dma_start(out=wo2_sb[:], in_=wo[P : P + D, :])
        nc.gpsimd.memset(ones_bf[:], 1.0)

        def qT(h, base=0):
            # h0 rows 0:64 of tA, h1 rows 64:128 of tA, h2 rows 64:128 of tD
            return (tA[0:D], tA[D:P], tD[D:P])[h]

        def kT(h, base=0):
            return (tB[0:D], tB[D:P], tE[D:P])[h]

        # psum budget (8 banks): p1 2 + scores 2*2 + outT 2
        with (
            tc.tile_pool(name="xchunks", bufs=2) as xpool,
            tc.tile_pool(name="p1psum", bufs=2, space="PSUM") as p1psum,
            tc.tile_pool(name="spsum", bufs=4 // GK, space="PSUM") as spool,
            tc.tile_pool(name="opsum", bufs=2, space="PSUM") as opool,
            tc.tile_pool(name="exps", bufs=3) as epool,
            tc.tile_pool(name="smalls", bufs=4) as rpool,
        ):
            from collections import deque

            def proj_work(qb):
                """Closures emitting projection chunk qb (tokens qb*512..)."""
                qsl = slice(qb * QB, (qb + 1) * QB)
                st = {}

                def dma():
                    xt = xpool.tile([P, 6, QB], BF16, tag="xt")
                    nc.sync.dma_start(
                        out=xt[:],
                        in_=xT[:, qsl].rearrange("(co p) t -> p co t", p=P),
                    )
                    st["xt"] = xt

                def chain(ci):
                    def f():
                        ps = p1psum.tile(
                            [P, QB], F32, tag="p1", name=f"p1_{qb}_{ci}"
                        )
                        for c6 in range(6):
                            nc.tensor.matmul(
                                ps[:],
                                wqk_sb[:, c6, ci * P : (ci + 1) * P],
                                st["xt"][:, c6, :],
                                start=(c6 == 0),
                                stop=(c6 == 5),
                            )
                        if ci < 2:
                            nc.vector.tensor_copy(
                                out=(tA, tB)[ci][:, qsl], in_=ps[:]
                            )
                        else:
                            # [q2|k2]: k2 (rows 64:) straight to tE; q2 must
                            # cross partitions - stage + DMA bounce
                            nc.vector.tensor_copy(
                                out=tE[D:P, qsl], in_=ps[D:P, :]
                            )
                            stg = rpool.tile([D, QB], BF16, tag="stg",
                                             bufs=2, name=f"stg_{qb}")
                            nc.vector.tensor_copy(out=stg[:], in_=ps[0:D, :])
                            nc.sync.dma_start(out=tD[D:P, qsl], in_=stg[:])
                    return f

                def vhalf(half):
                    def f():
                        ktv = qb * (QB // P) + half
                        ps2 = p1psum.tile(
                            [P, QB], F32, tag="p1", name=f"p1v_{qb}_{half}"
                        )
                        for c6 in range(6):
                            nc.tensor.matmul(
                                ps2[:, 0:256],
                                st["xt"][:, c6, half * P : (half + 1) * P],
                                wv_sb[:, c6, :],
                                start=(c6 == 0),
                                stop=(c6 == 5),
                            )
                        nc.vector.tensor_copy(
                            out=vaug[:, ktv, :], in_=ps2[:, 0:256]
                        )
                        if half == QB // P - 1:
                            # restore the ones columns the v copies overwrote
                            for h in range(HPC):
                                nc.vector.tensor_copy(
                                    out=vaug[:, qb * (QB // P) :
                                             (qb + 1) * (QB // P),
                                             h * (D + 1) + D],
                                    in_=ones_bf[:, 0 : QB // P],
                                )
                    return f

                return (
                    [dma]
                    + [chain(ci) for ci in range(3)]
                    + [vhalf(h) for h in range(QB // P)]
                )

            def outproj_work(qb):
                """Closures emitting the output projection of q block qb."""
                def tt_work(tt):
                    def f():
                        tsl = slice(tt * P, (tt + 1) * P)
                        so = rpool.tile([P, C], BF16, tag="p3out", bufs=2)
                        for noff, nsz in ((0, 512), (512, 256)):
                            po = p1psum.tile(
                                [P, QB], F32, tag="p1", name=f"po_{tt}_{noff}"
                            )
                            nc.tensor.matmul(
                                po[:, :nsz],
                                attnOAB[:, tsl],
                                woAB_sb[:, noff : noff + nsz],
                                start=True,
                                stop=False,
                            )
                            nc.tensor.matmul(
                                po[:, :nsz],
                                attnO2[:, tsl],
                                wo2_sb[:, noff : noff + nsz],
                                start=False,
                                stop=True,
                            )
                            nc.vector.tensor_copy(
                                out=so[:, noff : noff + nsz], in_=po[:, :nsz]
                            )
                        nc.sync.dma_start(out=out[tsl, :], in_=so[:])
                    return f

                return [
                    tt_work(tt)
                    for tt in range(qb * (QB // P), (qb + 1) * (QB // P))
                ]

            def attn_group(qb, g, hb, outps):
                """Two k-tiles (2g, 2g+1) for one or two heads: interleaved
                scores (consecutive matmuls always hit alternating PE
                row-groups, which run concurrently; same-row-group
                back-to-back K=64 matmuls serialize their weight loads and
                cost ~2.3x) -> one exp per head -> mask -> PV."""
                nkt = 4 * qb + 4
                hb = list(hb)
                sps, exs = [], []
                for h, _ in hb:
                    sps.append(spool.tile([P, GK * QB], F32, tag="sc",
                                          name=f"sp_{qb}_{h}_{g}"))
                for i in range(GK):
                    kt = GK * g + i
                    for (h, bases), sp in zip(hb, sps):
                        base = bases[i]
                        nc.tensor.matmul(
                            sp[:, i * QB : (i + 1) * QB],
                            kT(h, base)[:, kt * P : (kt + 1) * P],
                            qT(h, base)[:, qb * QB : (qb + 1) * QB],
                            start=True,
                            stop=True,
                        )
                for sp in sps:
                    ex = epool.tile([P, GK * QB], BF16, tag="ex")
                    nc.scalar.activation(
                        out=ex[:],
                        in_=sp[:],
                        func=mybir.ActivationFunctionType.Exp,
                        scale=float(D) ** -0.5,
                    )
                    exs.append(ex)
                for i in range(GK):
                    kt = GK * g + i
                    j = kt - 4 * qb
                    if j >= 0:  # diagonal tile: causal mask (fill 0 at q<k)
                        for ex in exs:
                            nc.gpsimd.affine_select(
                                out=ex[:, i * QB : (i + 1) * QB],
                                in_=ex[:, i * QB : (i + 1) * QB],
                                compare_op=mybir.AluOpType.is_ge,
                                fill=0.0,
                                base=-P * j,
                                pattern=[[1, QB]],
                                channel_multiplier=-1,
                            )
                for ((h, _), ex, outp) in zip(hb, exs, outps):
                    for i in range(GK):
                        kt = GK * g + i
                        co = max(0, P * (kt - 4 * qb))
                        nc.tensor.matmul(
                            outp[:, co:],
                            vaug[:, kt, h * (D + 1) : (h + 1) * (D + 1)],
                            ex[:, i * QB + co : (i + 1) * QB],
                            start=(kt == 0),
                            stop=(kt == nkt - 1),
                        )

            def normalize(qb, hs, outps):
                """Softmax denominators for one or two heads at once: DVE
                reciprocal of psum row 64, one sbuf->sbuf DMA to move the
                rows to partition 0 (partition_broadcast's ucode reads
                physical partition 0 only), one gpsimd broadcast, then the
                normalizing multiplies."""
                qsl = slice(qb * QB, (qb + 1) * QB)
                nh = len(hs)
                # copy PV psum to sbuf promptly so the opsum bank frees for
                # the next head's accumulation; normalize runs off sbuf
                ot = rpool.tile([D + 1, nh, QB], F32, tag=f"ot{nh}", bufs=2)
                for i, outp in enumerate(outps):
                    nc.vector.tensor_copy(out=ot[:, i, :], in_=outp[:])
                rt = rpool.tile([D + 1, nh, QB], F32, tag=f"recip{nh}", bufs=1)
                nc.vector.reciprocal(
                    out=rt[D : D + 1, :, :], in_=ot[D : D + 1, :, :]
                )
                rb = rpool.tile([1, nh, QB], F32, tag=f"rb{nh}", bufs=1)
                nc.sync.dma_start(out=rb[:], in_=rt[D : D + 1, :, :])
                rbc = rpool.tile([D, nh, QB], F32, tag=f"rbc{nh}", bufs=1)
                nc.gpsimd.partition_broadcast(rbc[:], rb[:])
                for i, h in enumerate(hs):
                    if h == 0:
                        dst = attnOAB[0:D, qsl]
                    elif h == 2:
                        dst = attnO2[:, qsl]
                    else:
                        # h1 belongs at partitions 64:128 of attnOAB, which
                        # engines cannot reach from lanes 0:64 - stage and
                        # DMA-bounce (sbuf->sbuf)
                        sg = rpool.tile([D, QB], BF16, tag="sg", bufs=2,
                                        name=f"sg_{qb}")
                        dst = sg[:]
                    nc.vector.tensor_mul(
                        out=dst, in0=ot[0:D, i, :], in1=rbc[:, i, :]
                    )
                    if h == 1:
                        nc.sync.dma_start(out=attnOAB[D:P, qsl], in_=dst)

            # Software pipeline by emission order: attention(qb) interleaves
            # closures of outproj(qb-1) and proj(qb+1) between its groups so
            # the PE stream stays dense and no sequencer head-of-line blocks.
            for w in proj_work(0):
                w()
            pending = deque(proj_work(1) if NQB > 1 else [])
            for qb in range(NQB):
                nkt = 4 * qb + 4
                # h0 (base 0) and h1 (base 64) interleaved for PE row-group
                # concurrency; h2 follows solo with alternating base
                outp0 = opool.tile([D + 1, QB], F32, tag="outT", name=f"o0_{qb}")
                outp1 = opool.tile([D + 1, QB], F32, tag="outT", name=f"o1_{qb}")
                for g in range(nkt // GK):
                    attn_group(qb, g, [(0, (0, 0)), (1, (0, 0))],
                               [outp0, outp1])
                    if pending:
                        pending.popleft()()
                normalize(qb, (0, 1), (outp0, outp1))
                outp2 = opool.tile([D + 1, QB], F32, tag="outT", name=f"o2_{qb}")
                for g in range(nkt // GK):
                    attn_group(qb, g, [(2, (D, D))], [outp2])
                    if pending:
                        pending.popleft()()
                normalize(qb, (2,), (outp2,))
                while pending:
                    pending.popleft()()
                pending = deque(outproj_work(qb))
                if qb + 2 < NQB:
                    pending.extend(proj_work(qb + 2))
            while pending:
                pending.popleft()()


def _get_nc():
    if "nc" not in _CACHE:
        _CACHE["nc"] = _build_nc()
    return _CACHE["nc"]


def _shard_inputs(x, w_qkv, w_out):
    """Build per-core input maps."""
    x = np.asarray(x, dtype=np.float32)
    w_qkv = np.asarray(w_qkv, dtype=np.float32)
    w_out = np.asarray(w_out, dtype=np.float32)
    xTs = [np.ascontiguousarray(x[b].T) for b in range(B)]
    in_maps = []
    for c in range(NCORES):
        b = c // 4
        heads = [HPC * (c % 4) + i for i in range(HPC)]
        q = [w_qkv[:, h * D : (h + 1) * D] for h in heads]
        k = [w_qkv[:, C + h * D : C + (h + 1) * D] for h in heads]
        wqk = np.concatenate([q[0], q[1], k[0], k[1], q[2], k[2]], axis=1)
        wv = np.zeros((C, 256), dtype=np.float32)
        for i, h in enumerate(heads):
            wv[:, i * (D + 1) : i * (D + 1) + D] = w_qkv[
                :, 2 * C + h * D : 2 * C + (h + 1) * D
            ]
        wo = np.concatenate(
            [w_out[h * D : (h + 1) * D, :] for h in heads], axis=0
        )  # [HPC*D, C]
        in_maps.append(
            {
                "xT": xTs[b].astype(bfloat16),
                "wqk": np.ascontiguousarray(wqk).astype(bfloat16),
                "wv": wv.astype(bfloat16),
                "wo": np.ascontiguousarray(wo).astype(bfloat16),
            }
        )
    return in_maps


def kernel(x, w_qkv, w_out, b_out):
    nc = _get_nc()
    in_maps = _shard_inputs(x, w_qkv, w_out)
    res = run_bass_kernel_spmd(nc, in_maps, core_ids=list(range(NCORES)))
    b_out = np.asarray(b_out, dtype=np.float32)
    outs = []
    for b in range(B):
        acc = res.results[4 * b]["out"].astype(np.float32).copy()
        for c in range(4 * b + 1, 4 * b + 4):
            acc += res.results[c]["out"]
        outs.append(acc + b_out[None, :])
    return np.stack(outs, axis=0)
